# revision 1
# baseline (speedup 1.0000x reference)
"""Trainium2 Bass kernel for nn_Expansion (e3nn-style tensor-product expansion).

Math reformulation (verified against the jax reference to ~1e-6):
  h   = silu(node_emb @ lw1 + lb1)                         [B,64]
  hb  = silu(node_emb @ bw1 + bb1)                         [B,64]
  x0  = feat[:,:128] @ W0 / sqrt(128)                      [B,16]
  x1k = feat[:,128+k::3] @ W1 / 8          (k=0,1,2)       [B,16]

The per-sample path contractions  r = sum_w w_path[b,w,:] * x[b,w]  with
w_path = (h @ lw2 + lb2) sliced, are a batched bilinear form

  r[b,p] = sum_{c,w} h'[b,c] x[b,w] M[(c,w), p],   h' = [h, 1]

which becomes a plain matmul over the outer product  z[b,(c,w)] = h'[b,c]*x[b,w]
(K = 65*16 = 1040) against reshaped weight matrices M built from lw2/lb2 on the
host.  This avoids materializing w = h@lw2 ([B,36864], ~600 MB) entirely.

Sharding: pure data parallel, batch 4096 -> 8 cores x 512.  Weights replicated.

Device layout per core (B_c = 512):
  - Inputs are passed pre-transposed ([feat_cols, B_c]) so the contraction dim
    lands on SBUF partitions with no on-device transposes.
  - z is built as 8 K-chunk tiles [128, 512] per z-type (4 types: x0, x1k) via
    DVE multiplies of partition-replicated h' and x tiles; the replication runs
    on the TensorEngine against constant 0/1 selection matrices (Gsel/Tsel),
    landing in SBUF as bf16 via ScalarEngine copies.
  - Main matmuls: out[b_tile=128, N<=512] accumulated over 9 K-chunks (+ one
    65-row bias-MLP chunk for the blk00/blk11 banks) into PSUM.
  - PSUM blocks are copied into an assembled [128, 80*80] SBUF tile with
    strided APs (the 1o interleave), then DMA'd contiguously to HBM.
All path normalization constants are folded into the host-side weight prep.
"""

import sys

import numpy as np

sys.path.insert(0, "/opt/trn_rl_repo")

import ml_dtypes  # noqa: E402

B_TOTAL = 4096
N_CORES = 8
BC = B_TOTAL // N_CORES  # 512 samples per core
P = 128
NB = BC // P  # 4 b-tiles per core
C3 = 1.0 / np.sqrt(3.0)

# matmul dtype mode: "bf16" | "f32r" | "f32"
MM_MODE = "bf16"

_CACHE = {}


def _np_mm_dtype(mode):
    return ml_dtypes.bfloat16 if mode == "bf16" else np.float32


def _build_program(mode, skip_lb2):
    import concourse.tile as tile
    from concourse import bacc, mybir

    F32 = mybir.dt.float32
    MM = mybir.dt.bfloat16 if mode == "bf16" else mybir.dt.float32
    AF = mybir.ActivationFunctionType

    def mmc(ap):
        # reinterpret f32 operands as float32r at the matmul for the fast path
        if mode == "f32r":
            return ap.bitcast(mybir.dt.float32r)
        return ap

    nc = bacc.Bacc("TRN2", target_bir_lowering=False, debug=False,
                   num_devices=N_CORES)

    t = {}
    t["featT"] = nc.dram_tensor("featT", [320, BC], F32, kind="ExternalInput").ap()
    t["node_embT"] = nc.dram_tensor("node_embT", [P, BC], F32, kind="ExternalInput").ap()
    t["W0"] = nc.dram_tensor("W0", [P, 16], F32, kind="ExternalInput").ap()
    t["W1"] = nc.dram_tensor("W1", [64, 16], F32, kind="ExternalInput").ap()
    t["lw1"] = nc.dram_tensor("lw1", [P, 64], F32, kind="ExternalInput").ap()
    t["bw1"] = nc.dram_tensor("bw1", [P, 64], F32, kind="ExternalInput").ap()
    t["lb1c"] = nc.dram_tensor("lb1c", [64, 1], F32, kind="ExternalInput").ap()
    t["bb1c"] = nc.dram_tensor("bb1c", [64, 1], F32, kind="ExternalInput").ap()
    t["R0"] = nc.dram_tensor("R0", [1040, 1280], MM, kind="ExternalInput").ap()
    t["R1"] = nc.dram_tensor("R1", [1040, 1024], MM, kind="ExternalInput").ap()
    t["BB"] = nc.dram_tensor("BB", [65, 1280], MM, kind="ExternalInput").ap()
    t["Gsel"] = nc.dram_tensor("Gsel", [65, 1024], MM, kind="ExternalInput").ap()
    t["Tsel"] = nc.dram_tensor("Tsel", [16, 128], MM, kind="ExternalInput").ap()
    t["out"] = nc.dram_tensor("out", [BC, 6400], F32, kind="ExternalOutput").ap()

    with tile.TileContext(nc) as tc:
        _emit(tc, t, mode, skip_lb2, mybir, MM, F32, AF, mmc)

    nc.compile()
    return nc


def _emit(tc, t, mode, skip_lb2, mybir, MM, F32, AF, mmc):
    nc = tc.nc
    from contextlib import ExitStack

    with ExitStack() as ctx:
        wpool = ctx.enter_context(tc.tile_pool(name="weights", bufs=1))
        apool = ctx.enter_context(tc.tile_pool(name="acts", bufs=1))
        zpool = ctx.enter_context(tc.tile_pool(name="z", bufs=1))
        opool = ctx.enter_context(tc.tile_pool(name="outs", bufs=3))
        pre_psum = ctx.enter_context(tc.tile_pool(name="pre_psum", bufs=1, space="PSUM"))
        prex_psum = ctx.enter_context(tc.tile_pool(name="prex_psum", bufs=2, space="PSUM"))
        main_psum = ctx.enter_context(tc.tile_pool(name="main_psum", bufs=5, space="PSUM"))

        # ---- weights / inputs to SBUF ----
        # small, latency-critical inputs first (they gate the prep chain)
        R0_sb = wpool.tile([P, 9, 1280], MM, tag="R0")
        R1_sb = wpool.tile([P, 9, 1024], MM, tag="R1")
        BB_sb = wpool.tile([65, 1280], MM, tag="BB")
        W0_sb = wpool.tile([P, 16], F32, tag="W0")
        W1_sb = wpool.tile([64, 16], F32, tag="W1")
        lw1_sb = wpool.tile([P, 64], F32, tag="lw1")
        bw1_sb = wpool.tile([P, 64], F32, tag="bw1")
        lb1_sb = wpool.tile([64, 1], F32, tag="lb1")
        bb1_sb = wpool.tile([64, 1], F32, tag="bb1")
        G_sb = wpool.tile([65, 1024], MM, tag="Gsel")
        T_sb = wpool.tile([16, 128], MM, tag="Tsel")

        feats_sb = apool.tile([P, BC], F32, tag="feats")
        featv_sb = [apool.tile([64, BC], F32, name=f"featv{k}", tag=f"featv{k}")
                    for k in range(3)]
        emb_sb = apool.tile([P, BC], F32, tag="emb")
        nc.sync.dma_start(emb_sb[:], t["node_embT"][:])
        nc.sync.dma_start(feats_sb[:], t["featT"][0:128])
        for k in range(3):
            nc.sync.dma_start(featv_sb[k][:], t["featT"][128 + 64 * k:192 + 64 * k])
        nc.sync.dma_start(lw1_sb[:], t["lw1"][:])
        nc.sync.dma_start(bw1_sb[:], t["bw1"][:])
        nc.sync.dma_start(W0_sb[:], t["W0"][:])
        nc.sync.dma_start(W1_sb[:], t["W1"][:])
        nc.sync.dma_start(lb1_sb[:], t["lb1c"][:])
        nc.sync.dma_start(bb1_sb[:], t["bb1c"][:])
        nc.sync.dma_start(G_sb[:], t["Gsel"][:])
        nc.sync.dma_start(T_sb[:], t["Tsel"][:])
        nc.sync.dma_start(BB_sb[:], t["BB"][:])

        # big weight matrices, split by the column blocks the matmul banks
        # consume, so the first banks can start before the full load lands
        r0v = t["R0"][0:1024].rearrange("(q p) n -> p q n", p=P)
        r1v = t["R1"][0:1024].rearrange("(q p) n -> p q n", p=P)
        for c0, c1 in ((0, 512), (512, 1024), (1024, 1280)):
            nc.sync.dma_start(R0_sb[:, 0:8, c0:c1], r0v[:, :, c0:c1])
        for c0, c1 in ((0, 512), (512, 1024)):
            nc.sync.dma_start(R1_sb[:, 0:8, c0:c1], r1v[:, :, c0:c1])
        if not skip_lb2:
            nc.sync.dma_start(R0_sb[0:16, 8, :], t["R0"][1024:1040])
            nc.sync.dma_start(R1_sb[0:16, 8, :], t["R1"][1024:1040])

        # ---- tiny MLP heads: hT, hbT, x0T, x1kT (all [*, BC] with contraction
        #      on partitions) ----
        ph = pre_psum.tile([64, BC], F32, tag="pre")
        nc.tensor.matmul(ph[:], lhsT=lw1_sb[:], rhs=emb_sb[:], start=True, stop=True)
        hp_sb = apool.tile([65, BC], MM, tag="hp")
        nc.scalar.activation(hp_sb[0:64, :], ph[:], AF.Silu, bias=lb1_sb[:])
        nc.any.memset(hp_sb[64:65, :], 1.0)

        pb = pre_psum.tile([64, BC], F32, tag="pre")
        nc.tensor.matmul(pb[:], lhsT=bw1_sb[:], rhs=emb_sb[:], start=True, stop=True)
        hbp_sb = apool.tile([65, BC], MM, tag="hbp")
        nc.scalar.activation(hbp_sb[0:64, :], pb[:], AF.Silu, bias=bb1_sb[:])
        nc.any.memset(hbp_sb[64:65, :], 1.0)

        xs_sb = []
        for tdx in range(4):
            px = prex_psum.tile([16, BC], F32, tag="px")
            if tdx == 0:
                nc.tensor.matmul(px[:], lhsT=W0_sb[:], rhs=feats_sb[:],
                                 start=True, stop=True)
            else:
                nc.tensor.matmul(px[:], lhsT=W1_sb[:], rhs=featv_sb[tdx - 1][:],
                                 start=True, stop=True)
            xf = apool.tile([16, BC], MM, name=f"xf{tdx}", tag=f"xf{tdx}")
            nc.scalar.copy(xf[:], px[:])
            xs_sb.append(xf)
        xs_mm = xs_sb

        # ---- partition-replicated tiles for the z outer product ----
        # Both replications run on PE against constant selection matrices,
        # then land in SBUF (as MM dtype) via ACT copies:
        #   xbc[t][p, b] = x_t[p % 16, b]        (Tsel[w, m] = [m%16 == w])
        #   hbc[q][p, b] = h'[8q + p//16, b]     (Gsel[c, 128q+16c8+w] = [c==8q+c8])
        xbc = []
        for tdx in range(4):
            px_bc = prex_psum.tile([P, BC], F32, name=f"pxbc{tdx}", tag="px")
            nc.tensor.matmul(px_bc[:], lhsT=T_sb[:], rhs=xs_sb[tdx][:],
                             start=True, stop=True)
            xb = apool.tile([P, BC], MM, name=f"xbc{tdx}", tag=f"xbc{tdx}")
            nc.scalar.copy(xb[:], px_bc[:])
            xbc.append(xb)
        hbc = []
        for q in range(8):
            ph_bc = prex_psum.tile([P, BC], F32, name=f"phbc{q}", tag="px")
            nc.tensor.matmul(ph_bc[:], lhsT=G_sb[:, P * q:P * (q + 1)],
                             rhs=hp_sb[:], start=True, stop=True)
            hb = apool.tile([P, BC], MM, name=f"hbc{q}", tag=f"hbc{q}")
            nc.scalar.copy(hb[:], ph_bc[:])
            hbc.append(hb)
        # z[t][q][(c8,w), b] = h'[8q+c8, b] * x_t[w, b]   (MM x MM -> MM on DVE)
        # z-type-outer order matches the PSUM-bank consumption order below, so
        # the first accumulation group unblocks after 8 DVE ops, not 29.
        z = [[None] * 8 for _ in range(4)]
        for tdx in range(4):
            for q in range(8):
                zt = zpool.tile([P, BC], MM, name=f"z{tdx}_{q}", tag=f"z{tdx}_{q}")
                nc.vector.tensor_mul(out=zt[:], in0=hbc[q][:], in1=xbc[tdx][:])
                z[tdx][q] = zt

        # ---- main matmuls + output assembly ----
        def accum2(tdx, rhs_sb, col0, ncols, bias_cols, bsl, psum_ap):
            nmm = 8 + (0 if skip_lb2 else 1) + (1 if bias_cols is not None else 0)
            idx = 0
            for q in range(8):
                idx += 1
                nc.tensor.matmul(psum_ap,
                                 lhsT=mmc(z[tdx][q][:, bsl]),
                                 rhs=mmc(rhs_sb[:, q, col0:col0 + ncols]),
                                 start=(idx == 1), stop=(idx == nmm))
            if not skip_lb2:
                idx += 1
                nc.tensor.matmul(psum_ap,
                                 lhsT=mmc(xs_mm[tdx][:, bsl]),
                                 rhs=mmc(rhs_sb[0:16, 8, col0:col0 + ncols]),
                                 start=False, stop=(idx == nmm))
            if bias_cols is not None:
                idx += 1
                nc.tensor.matmul(psum_ap,
                                 lhsT=mmc(hbp_sb[:, bsl]),
                                 rhs=mmc(BB_sb[:, bias_cols[0]:bias_cols[1]]),
                                 start=False, stop=(idx == nmm))

        for j in range(NB):
            bsl = slice(P * j, P * (j + 1))
            out_t = opool.tile([P, 6400], F32, name="out_t", tag="out_t")
            o3 = out_t.rearrange("p (r c) -> p r c", c=80)          # [128,80,80]
            top = o3[:, 0:32, :]                                     # [128,32,80]
            bot = out_t[:, 2560:6400].rearrange(
                "p (u i c) -> p u i c", i=3, c=80)                   # [128,16,3,80]

            # blk11 off-diagonal zeros
            nc.gpsimd.memset(o3[:, 32:80, 32:80], 0.0)

            # r00 -> blk00 (rows 0..31, cols 0..31), scale folded on host
            p00a = main_psum.tile([P, 512], F32, name="p00a", tag="mp")
            accum2(0, R0_sb, 0, 512, (0, 512), bsl, p00a[:])
            nc.scalar.copy(o3[:, 0:16, 0:32],
                           p00a[:].rearrange("p (u v) -> p u v", v=32))
            p00b = main_psum.tile([P, 512], F32, name="p00b", tag="mp")
            accum2(0, R0_sb, 512, 512, (512, 1024), bsl, p00b[:])
            nc.scalar.copy(o3[:, 16:32, 0:32],
                           p00b[:].rearrange("p (u v) -> p u v", v=32))

            # r11 -> blk11 diagonal-in-(i,j): out[32+3u+i, 32+3v+i]
            p11 = main_psum.tile([P, 512], F32, name="p11", tag="mp")
            accum2(0, R0_sb, 1024, 256, (1024, 1280), bsl, p11[:, 0:256])
            src11 = p11[:, 0:256].rearrange("p (u v) -> p u v", v=16)
            for i in range(3):
                dst = bot[:, :, i, 32:80].rearrange(
                    "p u (v jj) -> p u v jj", jj=3)[:, :, :, i]      # [128,16,16]
                nc.vector.tensor_copy(out=dst, in_=src11)

            # r01k -> blk01: out[u, 32+3v+k], u<32, v<16
            for k in range(3):
                p01 = main_psum.tile([P, 512], F32, name=f"p01_{k}", tag="mp")
                accum2(1 + k, R1_sb, 0, 512, None, bsl, p01[:])
                dst = top[:, :, 32:80].rearrange(
                    "p u (v jj) -> p u v jj", jj=3)[:, :, :, k]      # [128,32,16]
                src = p01[:].rearrange("p (u v) -> p u v", v=16)
                if k == 0:
                    nc.scalar.copy(dst, src)
                else:
                    nc.vector.tensor_copy(out=dst, in_=src)

            # r10i -> blk10: out[32+3u+i, v], u<16, v<32
            for i in range(3):
                p10 = main_psum.tile([P, 512], F32, name=f"p10_{i}", tag="mp")
                accum2(1 + i, R1_sb, 512, 512, None, bsl, p10[:])
                dst = bot[:, :, i, 0:32]                             # [128,16,32]
                src = p10[:].rearrange("p (u v) -> p u v", v=32)
                if i == 0:
                    nc.scalar.copy(dst, src)
                else:
                    nc.vector.tensor_copy(out=dst, in_=src)

            # split the writeback so the top half (blk00|blk01) can leave
            # while the bottom half (blk10|blk11) is still being assembled
            nc.sync.dma_start(t["out"][bsl, 0:2560], out_t[:, 0:2560])
            nc.sync.dma_start(t["out"][bsl, 2560:6400], out_t[:, 2560:6400])


def _prepare(inputs, mode):
    f32 = np.float32
    feat = np.ascontiguousarray(np.asarray(inputs["feat"], dtype=f32))
    node_emb = np.ascontiguousarray(np.asarray(inputs["node_emb"], dtype=f32))
    W0 = np.asarray(inputs["W0"], f32)
    W1 = np.asarray(inputs["W1"], f32)
    lw1 = np.asarray(inputs["lw1"], f32)
    lb1 = np.asarray(inputs["lb1"], f32)
    lw2 = np.asarray(inputs["lw2"], f32)
    lb2 = np.asarray(inputs["lb2"], f32)
    bw1 = np.asarray(inputs["bw1"], f32)
    bb1 = np.asarray(inputs["bb1"], f32)
    bw2 = np.asarray(inputs["bw2"], f32)
    bb2 = np.asarray(inputs["bb2"], f32)

    mmnp = _np_mm_dtype(mode)
    s16 = np.float32(1.0 / 16.0)
    sC = np.float32(C3 / 16.0)

    lw2p = np.concatenate([lw2, lb2[None]], axis=0)           # [65, 36864]
    M00 = lw2p[:, :16384].reshape(1040, 1024) * s16
    M11 = lw2p[:, 16384:20480].reshape(1040, 256) * sC
    M01 = lw2p[:, 20480:28672].reshape(1040, 512) * sC
    M10 = lw2p[:, 28672:36864].reshape(1040, 512) * sC
    R0 = np.ascontiguousarray(np.concatenate([M00, M11], axis=1)).astype(mmnp)
    R1 = np.ascontiguousarray(np.concatenate([M01, M10], axis=1)).astype(mmnp)
    BBf = np.concatenate([bw2, bb2[None]], axis=0)            # [65, 1280]
    BB = np.ascontiguousarray(
        np.concatenate([BBf[:, :1024] * s16, BBf[:, 1024:] * sC], axis=1)
    ).astype(mmnp)

    W0s = np.ascontiguousarray(W0 * np.float32(1.0 / np.sqrt(128.0)))
    W1s = np.ascontiguousarray(W1 * np.float32(1.0 / 8.0))
    lb1c = np.ascontiguousarray(lb1[:, None])
    bb1c = np.ascontiguousarray(bb1[:, None])

    # selection matrices for the PE-based partition replications
    Gsel = np.zeros((65, 1024), np.float32)
    for q in range(8):
        for c8 in range(8):
            Gsel[8 * q + c8, 128 * q + 16 * c8:128 * q + 16 * (c8 + 1)] = 1.0
    Tsel = np.zeros((16, 128), np.float32)
    for w in range(16):
        Tsel[w, w::16] = 1.0
    Gsel = Gsel.astype(mmnp)
    Tsel = Tsel.astype(mmnp)

    skip_lb2 = not bool(np.any(lb2))

    in_maps = []
    for i in range(N_CORES):
        sl = slice(i * BC, (i + 1) * BC)
        fs = feat[sl]
        featT = np.ascontiguousarray(
            np.concatenate(
                [fs[:, :128], fs[:, 128::3], fs[:, 129::3], fs[:, 130::3]],
                axis=1).T)                                     # [320, BC]
        embT = np.ascontiguousarray(node_emb[sl].T)            # [128, BC]
        in_maps.append({
            "featT": featT,
            "node_embT": embT,
            "W0": W0s, "W1": W1s,
            "lw1": lw1, "bw1": bw1,
            "lb1c": lb1c, "bb1c": bb1c,
            "R0": R0, "R1": R1, "BB": BB, "Gsel": Gsel, "Tsel": Tsel,
        })
    return in_maps, skip_lb2


def run(inputs, mode=None, trace=False):
    """Build (cached), run on 8 cores, gather. Returns (out, results)."""
    mode = mode or MM_MODE
    in_maps, skip_lb2 = _prepare(inputs, mode)
    key = (mode, skip_lb2)
    if key not in _CACHE:
        _CACHE[key] = _build_program(mode, skip_lb2)
    nc = _CACHE[key]

    from concourse.bass_utils import run_bass_kernel_spmd
    res = run_bass_kernel_spmd(nc, in_maps, list(range(N_CORES)), trace=trace)
    out = np.concatenate(
        [res.results[i]["out"].reshape(BC, 80, 80) for i in range(N_CORES)],
        axis=0)
    return out.astype(np.float32), res


def kernel(**inputs):
    out, _ = run(inputs)
    return out



# revision 33
# speedup vs baseline: 1.2143x; 1.2143x over previous
"""Trainium2 Bass kernel for nn_Expansion (e3nn-style tensor-product expansion).

Math reformulation (verified against the jax reference):
  h   = silu(node_emb @ lw1 + lb1)                         [B,64]
  hb  = silu(node_emb @ bw1 + bb1)                         [B,64]
  x0  = feat[:,:128] @ W0 / sqrt(128)                      [B,16]
  x1k = feat[:,128+k::3] @ W1 / 8          (k=0,1,2)       [B,16]

The per-sample path contractions with w_path = (h @ lw2 + lb2) sliced are a
batched bilinear form

  r[b,p] = sum_{c,w} h'[b,c] x[b,w] M[(c,w), p],   h' = [h, 1]

i.e. a plain matmul over the outer product  z[b,(c,w)] = h'[b,c]*x[b,w]
(K = 65*16 = 1040; the c=64 block is lb2 and is skipped when lb2 == 0)
against reshaped weight matrices M built from lw2/lb2 on the host.  This
avoids materializing w = h@lw2 ([B,36864], ~600 MB) entirely.

Sharding: pure data parallel, batch 4096 -> 8 cores x 512.  Weights replicated.

Device layout per core (B_c = 512):
  - All activations load as bf16 with the contraction dim on partitions.
  - The partition-replicated tiles the z outer product needs are produced
    DIRECTLY by the pre-matmuls: host-side column-replicated weights
    (LW1R = lw1[:, repeat], W0R/W1R = tile(W,8)) make the PE emit
    hbc[q][p,b] = h_pre[8q+p//16, b] and xbc[t][p,b] = x_t[p%16, b]; the
    SiLU (with replicated per-partition bias) and PSUM->SBUF casts land
    them in SBUF as bf16.
  - z tiles (DVE bf16 multiplies) feed the main matmuls: out[128b, N<=512]
    accumulated over 8 K-chunks (+ 65-row bias-MLP chunk for blk00/blk11).
  - A burst of dummy warm-up matmuls at t=0 ramps the PE clock out of its
    cold p-state before the real work arrives.
  - Big weight matrices stream via the Pool/SWDGE DMA path so the critical
    activation loads own the HWDGE queue; output writes are split into
    1280-column chunks so the writeback pipelines tightly with compute.
All path normalization constants are folded into the host-side weight prep.
"""

import sys

import numpy as np

sys.path.insert(0, "/opt/trn_rl_repo")

import ml_dtypes  # noqa: E402

B_TOTAL = 4096
N_CORES = 8
BC = B_TOTAL // N_CORES  # 512 samples per core
P = 128
NB = BC // P  # 4 b-tiles per core
C3 = 1.0 / np.sqrt(3.0)

MM_MODE = "bf16"
N_WARM = 10

_CACHE = {}


def _build_program(mode, skip_lb2):
    import concourse.tile as tile
    from concourse import bacc, mybir

    F32 = mybir.dt.float32
    MM = mybir.dt.bfloat16
    AF = mybir.ActivationFunctionType

    nc = bacc.Bacc("TRN2", target_bir_lowering=False, debug=False,
                   num_devices=N_CORES)

    t = {}
    # blob16: [emb 512 | lw1r_q0 128 | lb1r 8 | bb1 1 || feats 512 |
    #          W0R 128 | lw1r_q1:8 896 || W1R(pad) 128 | BW1 64]
    # -> [128, 2377] bf16; the first 649 columns alone unlock the hbc0
    # matmul + SiLU, the next 640 the xbc0 matmul
    t["blob16"] = nc.dram_tensor("blob16", [P, 2377], MM, kind="ExternalInput").ap()
    t["featv"] = nc.dram_tensor("featv", [64, 3, BC], MM, kind="ExternalInput").ap()
    t["R0"] = nc.dram_tensor("R0", [1040, 1280], MM, kind="ExternalInput").ap()
    t["R1"] = nc.dram_tensor("R1", [1040, 1024], MM, kind="ExternalInput").ap()
    t["BB"] = nc.dram_tensor("BB", [65, 1280], MM, kind="ExternalInput").ap()
    t["out"] = nc.dram_tensor("out", [BC, 6400], F32, kind="ExternalOutput").ap()

    with tile.TileContext(nc) as tc:
        _emit(tc, t, skip_lb2, mybir, MM, F32, AF)

    nc.compile()
    return nc


def _emit(tc, t, skip_lb2, mybir, MM, F32, AF):
    nc = tc.nc
    from contextlib import ExitStack

    with ExitStack() as ctx:
        wpool = ctx.enter_context(tc.tile_pool(name="weights", bufs=1))
        apool = ctx.enter_context(tc.tile_pool(name="acts", bufs=1))
        zpool = ctx.enter_context(tc.tile_pool(name="z", bufs=1))
        opool = ctx.enter_context(tc.tile_pool(name="outs", bufs=3))
        prex_psum = ctx.enter_context(tc.tile_pool(name="prex_psum", bufs=4, space="PSUM"))
        main_psum = ctx.enter_context(tc.tile_pool(name="main_psum", bufs=4, space="PSUM"))

        # ---- PE warm-up: ramp the clock out of the cold p-state while the
        #      input DMAs are still in flight ----
        wsrc = wpool.tile([P, 256], MM, tag="wsrc")
        nc.vector.memset(wsrc[:], 0.0)
        wp = prex_psum.tile([64, 256], F32, name="warm", tag="px")
        for _ in range(N_WARM):
            nc.tensor.matmul(wp[:], lhsT=wsrc[:, 0:64], rhs=wsrc[:],
                             start=True, stop=True)

        # ---- weights / inputs to SBUF ----
        # critical-path activation loads in one packed DMA each (HWDGE
        # descriptor-gen costs ~0.6us per DMA, so fewer+bigger wins); the
        # big matmul weights stream via Pool/SWDGE off the HWDGE queue
        # three separate SBUF tiles so tile-granularity dependency tracking
        # doesn't make early consumers wait on later chunks
        blobA = wpool.tile([P, 649], MM, tag="blobA")
        blobB = wpool.tile([P, 1536], MM, tag="blobB")
        blobC = wpool.tile([P, 192], MM, tag="blobC")
        featv_sb = apool.tile([64, 3, BC], MM, tag="featv")
        BB_sb = wpool.tile([65, 1280], MM, tag="BB")
        R0_sb = wpool.tile([P, 9, 1280], MM, tag="R0")
        R1_sb = wpool.tile([P, 9, 1024], MM, tag="R1")

        emb_sb = blobA[:, 0:512]
        lb1r_sb = blobA[:, 640:648]
        bb1_sb = blobA[0:64, 648:649]
        feats_sb = blobB[:, 0:512]
        w0r_sb = blobB[:, 512:640]
        w1r_sb = blobC[0:64, 0:128]
        bw1_sb = blobC[:, 128:192]

        def lw1r_q(q):
            # q0 rides in the first blob chunk; q1..7 in the second
            if q == 0:
                return blobA[:, 512:640]
            return blobB[:, 640 + P * (q - 1):640 + P * q]

        nc.sync.dma_start(blobA[:], t["blob16"][:, 0:649])
        nc.sync.dma_start(blobB[:], t["blob16"][:, 649:2185])
        nc.sync.dma_start(blobC[:], t["blob16"][:, 2185:2377])
        nc.sync.dma_start(featv_sb[:], t["featv"][:])
        nc.sync.dma_start(BB_sb[:], t["BB"][:])

        r0v = t["R0"][0:1024].rearrange("(q p) n -> p q n", p=P)
        r1v = t["R1"][0:1024].rearrange("(q p) n -> p q n", p=P)
        # delay the SWDGE prefetch just long enough that the critical
        # activation loads win the DMA device, then stream the R chunks in
        # main-bank consumption order (p00a, p01*, p00b, p11, p10*);
        # R1c0/R0c1 go in q-halves so their banks start on the early half
        dly = wpool.tile([P, 780], MM, tag="dly")
        nc.gpsimd.memset(dly[:], 0.0)
        nc.gpsimd.dma_start(R0_sb[:, 0:8, 0:512], r0v[:, :, 0:512])
        nc.gpsimd.dma_start(R1_sb[:, 0:4, 0:512], r1v[:, 0:4, 0:512])
        nc.gpsimd.dma_start(R1_sb[:, 4:8, 0:512], r1v[:, 4:8, 0:512])
        nc.gpsimd.dma_start(R0_sb[:, 0:4, 512:1024], r0v[:, 0:4, 512:1024])
        nc.gpsimd.dma_start(R0_sb[:, 4:8, 512:1024], r0v[:, 4:8, 512:1024])
        nc.gpsimd.dma_start(R1_sb[:, 0:8, 512:1024], r1v[:, :, 512:1024])
        nc.gpsimd.dma_start(R0_sb[:, 0:8, 1024:1280], r0v[:, :, 1024:1280])
        if not skip_lb2:
            nc.sync.dma_start(R0_sb[0:16, 8, :], t["R0"][1024:1040])
            nc.sync.dma_start(R1_sb[0:16, 8, :], t["R1"][1024:1040])

        # ---- prep: replicated h (SiLU) and x tiles straight off the PE ----
        hbc = [apool.tile([P, BC], MM, name=f"hbc{q}", tag=f"hbc{q}")
               for q in range(8)]
        xbc = [apool.tile([P, BC], MM, name=f"xbc{t_}", tag=f"xbc{t_}")
               for t_ in range(4)]
        hbp_sb = apool.tile([65, BC], MM, tag="hbp")

        def hbc_mm(q):
            ph = prex_psum.tile([P, BC], F32, name=f"phbc{q}", tag="px")
            nc.tensor.matmul(ph[:], lhsT=lw1r_q(q),
                             rhs=emb_sb[:], start=True, stop=True)
            nc.scalar.activation(hbc[q][:], ph[:], AF.Silu,
                                 bias=lb1r_sb[:, q:q + 1])

        def xbc_mm(tdx):
            px = prex_psum.tile([P, BC], F32, name=f"pxbc{tdx}", tag="px")
            if tdx == 0:
                nc.tensor.matmul(px[:], lhsT=w0r_sb[:], rhs=feats_sb[:],
                                 start=True, stop=True)
            else:
                nc.tensor.matmul(px[:], lhsT=w1r_sb[:],
                                 rhs=featv_sb[:, tdx - 1, :],
                                 start=True, stop=True)
            nc.vector.tensor_copy(out=xbc[tdx][:], in_=px[:])

        # hbc0 first (its operands arrive in the first blob chunk), then
        # xbc0; the featv-dependent xbc1..3 go AFTER the whole h chain so a
        # late featv can't block the in-order PE queue
        hbc_mm(0)
        xbc_mm(0)
        for q in range(1, 8):
            hbc_mm(q)

        # bias-MLP head hb' = [silu(emb@bw1+bb1), 1]
        pb = prex_psum.tile([64, BC], F32, name="phbp", tag="px")
        nc.tensor.matmul(pb[:], lhsT=bw1_sb[:], rhs=emb_sb[:],
                         start=True, stop=True)
        nc.scalar.activation(hbp_sb[0:64, :], pb[:], AF.Silu, bias=bb1_sb[:])
        nc.vector.memset(hbp_sb[64:65, :], 1.0)

        xbc_mm(1)
        xbc_mm(2)
        xbc_mm(3)

        # ---- z outer-product tiles (DVE bf16) ----
        # Sliced per b-tile (the main matmuls for tile j only read columns
        # [128j:128j+128]) and produced one tile ahead, so tile-0 banks
        # start ~4us earlier and later tiles never wait on DVE.
        z = [[zpool.tile([P, BC], MM, name=f"z{tdx}_{q}", tag=f"z{tdx}_{q}")
              for q in range(8)] for tdx in range(4)]

        def z_slice(j):
            bsl = slice(P * j, P * (j + 1))
            for tdx in range(4):
                for q in range(8):
                    nc.vector.tensor_mul(out=z[tdx][q][:, bsl],
                                         in0=hbc[q][:, bsl],
                                         in1=xbc[tdx][:, bsl])

        z_slice(0)

        # ---- main matmuls + output assembly ----
        def accum2(tdx, rhs_sb, col0, ncols, bias_cols, bsl, psum_ap):
            # bias and lb2 terms FIRST so the accumulation group (and with it
            # the PSUM->SBUF copy) closes on the last z chunk, not on a
            # trailing extra matmul
            nmm = 8 + (0 if skip_lb2 else 1) + (1 if bias_cols is not None else 0)
            idx = 0
            if bias_cols is not None:
                idx += 1
                nc.tensor.matmul(psum_ap,
                                 lhsT=hbp_sb[:, bsl],
                                 rhs=BB_sb[:, bias_cols[0]:bias_cols[1]],
                                 start=True, stop=False)
            if not skip_lb2:
                idx += 1
                nc.tensor.matmul(psum_ap,
                                 lhsT=xbc[tdx][0:16, bsl],
                                 rhs=rhs_sb[0:16, 8, col0:col0 + ncols],
                                 start=(idx == 1), stop=False)
            for q in range(8):
                idx += 1
                nc.tensor.matmul(psum_ap,
                                 lhsT=z[tdx][q][:, bsl],
                                 rhs=rhs_sb[:, q, col0:col0 + ncols],
                                 start=(idx == 1), stop=(idx == nmm))

        def emit_p01(j, out_t, o3, top, bsl):
            # r01k -> blk01: out[u, 32+3v+k], u<32, v<16
            for k in range(3):
                p01 = main_psum.tile([P, 512], F32, name=f"p01_{k}", tag="mp")
                accum2(1 + k, R1_sb, 0, 512, None, bsl, p01[:])
                dst = top[:, :, 32:80].rearrange(
                    "p u (v jj) -> p u v jj", jj=3)[:, :, :, k]      # [128,32,16]
                src = p01[:].rearrange("p (u v) -> p u v", v=16)
                if k == 0:
                    nc.scalar.copy(dst, src)
                else:
                    nc.vector.tensor_copy(out=dst, in_=src)

        def emit_p00(j, out_t, o3, bsl, which, fine_tail):
            # r00 -> blk00 rows 0..15 (a) / 16..31 (b), plus the top write
            # for those rows once the p01 columns are in place
            col0, bias, r0, wa, wb = (
                (0, (0, 512), 0, 0, 1280) if which == 0
                else (512, (512, 1024), 16, 1280, 2560))
            p00 = main_psum.tile([P, 512], F32, name=f"p00{which}", tag="mp")
            accum2(0, R0_sb, col0, 512, bias, bsl, p00[:])
            pv = p00[:].rearrange("p (u v) -> p u v", v=32)
            if fine_tail:
                # the tail writebacks: copy + DMA in row-halves so the
                # final transfers are 640 columns each
                wm = (wa + wb) // 2
                nc.scalar.copy(o3[:, r0:r0 + 8, 0:32], pv[:, 0:8])
                nc.sync.dma_start(t["out"][bsl, wa:wm], out_t[:, wa:wm])
                nc.scalar.copy(o3[:, r0 + 8:r0 + 16, 0:32], pv[:, 8:16])
                nc.sync.dma_start(t["out"][bsl, wm:wb], out_t[:, wm:wb])
            else:
                nc.scalar.copy(o3[:, r0:r0 + 16, 0:32], pv)
                nc.sync.dma_start(t["out"][bsl, wa:wb], out_t[:, wa:wb])

        def emit_top(j, out_t, o3, top, bsl, fine_tail=False):
            if fine_tail:
                # last tile: p01 banks first so the row-0:16 write overlaps
                # the final p00 banks
                emit_p01(j, out_t, o3, top, bsl)
                emit_p00(j, out_t, o3, bsl, 0, True)
                emit_p00(j, out_t, o3, bsl, 1, True)
            else:
                emit_p00a_then_p01(j, out_t, o3, top, bsl)

        def emit_p00a_then_p01(j, out_t, o3, top, bsl):
            p00a = main_psum.tile([P, 512], F32, name="p00a", tag="mp")
            accum2(0, R0_sb, 0, 512, (0, 512), bsl, p00a[:])
            nc.scalar.copy(o3[:, 0:16, 0:32],
                           p00a[:].rearrange("p (u v) -> p u v", v=32))
            emit_p01(j, out_t, o3, top, bsl)
            nc.sync.dma_start(t["out"][bsl, 0:1280], out_t[:, 0:1280])
            p00b = main_psum.tile([P, 512], F32, name="p00b", tag="mp")
            accum2(0, R0_sb, 512, 512, (512, 1024), bsl, p00b[:])
            nc.scalar.copy(o3[:, 16:32, 0:32],
                           p00b[:].rearrange("p (u v) -> p u v", v=32))
            nc.sync.dma_start(t["out"][bsl, 1280:2560], out_t[:, 1280:2560])

        def emit_bot(j, out_t, o3, bot, bsl):
            # r10i -> blk10: out[32+3u+i, v], u<16, v<32
            # (before r11 so the bank order matches R-chunk arrival order)
            for i in range(3):
                p10 = main_psum.tile([P, 512], F32, name=f"p10_{i}", tag="mp")
                accum2(1 + i, R1_sb, 512, 512, None, bsl, p10[:])
                dst = bot[:, :, i, 0:32]                             # [128,16,32]
                src = p10[:].rearrange("p (u v) -> p u v", v=32)
                if i == 1:
                    nc.vector.tensor_copy(out=dst, in_=src)
                else:
                    # i==0,2 on ACT: GPSIMD cannot read PSUM
                    nc.scalar.copy(dst, src)

            # r11 -> blk11 diagonal-in-(i,j): out[32+3u+i, 32+3v+i]
            p11 = main_psum.tile([P, 512], F32, name="p11", tag="mp")
            accum2(0, R0_sb, 1024, 256, (1024, 1280), bsl, p11[:, 0:256])
            src11 = p11[:, 0:256].rearrange("p (u v) -> p u v", v=16)
            for i in range(3):
                dst = bot[:, :, i, 32:80].rearrange(
                    "p u (v jj) -> p u v jj", jj=3)[:, :, :, i]      # [128,16,16]
                if i == 1:
                    nc.vector.tensor_copy(out=dst, in_=src11)
                else:
                    # i==0,2 on ACT: GPSIMD cannot read PSUM
                    nc.scalar.copy(dst, src11)

            # bottom half in three chunks for tight writeback pipelining
            nc.sync.dma_start(t["out"][bsl, 2560:3840], out_t[:, 2560:3840])
            nc.sync.dma_start(t["out"][bsl, 3840:5120], out_t[:, 3840:5120])
            nc.sync.dma_start(t["out"][bsl, 5120:6400], out_t[:, 5120:6400])

        for j in range(NB):
            bsl = slice(P * j, P * (j + 1))
            out_t = opool.tile([P, 6400], F32, name="out_t", tag="out_t")
            o3 = out_t.rearrange("p (r c) -> p r c", c=80)          # [128,80,80]
            top = o3[:, 0:32, :]                                     # [128,32,80]
            bot = out_t[:, 2560:6400].rearrange(
                "p (u i c) -> p u i c", i=3, c=80)                   # [128,16,3,80]

            # blk11 off-diagonal zeros
            nc.gpsimd.memset(o3[:, 32:80, 32:80], 0.0)

            # next tile's z slices build on DVE under this tile's matmuls
            if j + 1 < NB:
                z_slice(j + 1)

            if j < NB - 1:
                emit_top(j, out_t, o3, top, bsl)
                emit_bot(j, out_t, o3, bot, bsl)
            else:
                # last tile bottom-first: the tail then ends on the two
                # small top writes instead of the three bottom ones
                emit_bot(j, out_t, o3, bot, bsl)
                emit_top(j, out_t, o3, top, bsl, fine_tail=True)


def _prepare(inputs, mode):
    f32 = np.float32
    bf16 = ml_dtypes.bfloat16
    feat = np.ascontiguousarray(np.asarray(inputs["feat"], dtype=f32))
    node_emb = np.ascontiguousarray(np.asarray(inputs["node_emb"], dtype=f32))
    W0 = np.asarray(inputs["W0"], f32)
    W1 = np.asarray(inputs["W1"], f32)
    lw1 = np.asarray(inputs["lw1"], f32)
    lb1 = np.asarray(inputs["lb1"], f32)
    lw2 = np.asarray(inputs["lw2"], f32)
    lb2 = np.asarray(inputs["lb2"], f32)
    bw1 = np.asarray(inputs["bw1"], f32)
    bb1 = np.asarray(inputs["bb1"], f32)
    bw2 = np.asarray(inputs["bw2"], f32)
    bb2 = np.asarray(inputs["bb2"], f32)

    s16 = np.float32(1.0 / 16.0)
    sC = np.float32(C3 / 16.0)

    lw2p = np.concatenate([lw2, lb2[None]], axis=0)           # [65, 36864]
    M00 = lw2p[:, :16384].reshape(1040, 1024) * s16
    M11 = lw2p[:, 16384:20480].reshape(1040, 256) * sC
    M01 = lw2p[:, 20480:28672].reshape(1040, 512) * sC
    M10 = lw2p[:, 28672:36864].reshape(1040, 512) * sC
    R0 = np.ascontiguousarray(np.concatenate([M00, M11], axis=1)).astype(bf16)
    R1 = np.ascontiguousarray(np.concatenate([M01, M10], axis=1)).astype(bf16)
    BBf = np.concatenate([bw2, bb2[None]], axis=0)            # [65, 1280]
    BB = np.ascontiguousarray(
        np.concatenate([BBf[:, :1024] * s16, BBf[:, 1024:] * sC], axis=1)
    ).astype(bf16)

    W0s = W0 * np.float32(1.0 / np.sqrt(128.0))
    W1s = W1 * np.float32(1.0 / 8.0)

    # column-replicated pre-matmul weights: the PE emits partition-replicated
    # activations directly (see module docstring)
    rep = np.repeat(np.arange(64), 16)                        # c = j // 16
    LW1R = lw1[:, rep].astype(bf16)                           # [128, 1024]
    LB1R = np.empty((P, 8), f32)
    for q in range(8):
        LB1R[:, q] = lb1[8 * q + np.arange(P) // 16]
    W0R = np.tile(W0s, (1, 8)).astype(bf16)                   # [128, 128]
    W1R = np.zeros((P, P), bf16)
    W1R[0:64] = np.tile(W1s, (1, 8)).astype(bf16)             # [64, 128] + pad
    BW1 = bw1.astype(bf16)

    LB1Rb = LB1R.astype(bf16)                                 # [128, 8]
    bb1b = np.zeros((P, 1), bf16)
    bb1b[0:64, 0] = bb1.astype(bf16)

    skip_lb2 = not bool(np.any(lb2))

    in_maps = []
    for i in range(N_CORES):
        sl = slice(i * BC, (i + 1) * BC)
        fs = feat[sl]
        embT = node_emb[sl].T.astype(bf16)                    # [128, BC]
        featsT = fs[:, :128].T.astype(bf16)                   # [128, BC]
        featv = np.stack(
            [fs[:, 128 + k::3].T.astype(bf16) for k in range(3)], axis=1
        )                                                     # [64, 3, BC]
        blob16 = np.ascontiguousarray(np.concatenate(
            [embT, LW1R[:, 0:128], LB1Rb, bb1b, featsT, W0R,
             LW1R[:, 128:1024], W1R, BW1], axis=1))           # [128, 2377]
        in_maps.append({
            "blob16": blob16,
            "featv": np.ascontiguousarray(featv),
            "R0": R0, "R1": R1, "BB": BB,
        })
    return in_maps, skip_lb2


def run(inputs, mode=None, trace=False):
    """Build (cached), run on 8 cores, gather. Returns (out, results)."""
    mode = mode or MM_MODE
    in_maps, skip_lb2 = _prepare(inputs, mode)
    key = (mode, skip_lb2)
    if key not in _CACHE:
        _CACHE[key] = _build_program(mode, skip_lb2)
    nc = _CACHE[key]

    from concourse.bass_utils import run_bass_kernel_spmd
    res = run_bass_kernel_spmd(nc, in_maps, list(range(N_CORES)), trace=trace)
    out = np.concatenate(
        [res.results[i]["out"].reshape(BC, 80, 80) for i in range(N_CORES)],
        axis=0)
    return out.astype(np.float32), res


def kernel(**inputs):
    out, _ = run(inputs)
    return out
